# revision 1
# baseline (speedup 1.0000x reference)
"""Trainium2 Bass kernel for the L2-normalized attention module.

Reference computation (per batch b):
    qkv = x @ w_qkv.T                        # [n, 3*dim]
    q,k,v per head h (d=64)                  # [n, d]
    dots = q @ k.T                           # [n, n]
    attn = dots / max(||dots_row||_2, eps) * g + b
    out_h = attn @ v                         # [n, d]
    final = concat_h(out_h) @ w_out.T + b_out

Key algebraic factorization: the score "nonlinearity" is only a per-row
scale r_i = 1/max(||dots_i||, eps), and ||dots_i||^2 = q_i^T (k^T k) q_i.
With W = k^T v, G = k^T k:
    outT[:, i] = r_i * (W^T q_i),   r_i = rsqrt(q_i^T G q_i)
so the n x n score matrix is never materialized.

Sharding: 8 cores = 2 batches x 4 head-groups (4 heads each).  Each core
computes its qkv slice, the factored attention for its 4 heads, and a
partial w_out projection; the host sums the 4 partials per batch (TP
reduction) and adds b_out.  norm_g (and a 2^-6 Gram scale compensation)
is folded into w_out on the host; norm_b (zero in practice) gets an
exact host-side rank-1 correction.

Schedule (single PE stream, authored for zero tensor-engine gaps so the
HAM clock gate never falls back to half rate):
  - DMA-free warmup matmuls (memset SBUF operand) open the PE clock
    gate during the ~9us before the first input DMA completes (template
    preamble + DMA fixed latency).
  - phase A per 512-seq chunk: q projection, then kv projection; the
    Gram [G|W] accumulates in per-pair pinned PSUM banks across the
    whole phase (no vector adds, no phase barrier; interleaved
    accumulation groups must NOT share a bank — start=True clears the
    bank's has_written state).  The last chunk computes kv first so the
    final Gram stop + block-diag copies overlap its q matmuls.
  - phase B/C is software-pipelined per chunk: the t/rep/o matmuls of
    chunk c+1 are emitted between the w_out matmul groups of chunk c,
    with the Vector/Scalar ops (prods, rsqrt, rinv-mul, PSUM
    evacuations) balanced to stay under the PE's per-chunk time.
  - rinv uses the scalar-engine Rsqrt table in ONE pass (instead of
    Ln+Exp): rinv = rsqrt(s*norm^2 + s*eps^2) with s=2^-6 keeping the
    fp16 intermediates in range; the 1/sqrt(s) factor is folded into
    w_out on the host.
  - w_out partials evacuate as [128,1024] two-bank PSUM reads (cg tiles
    rotate 3-deep through the same slots the Gram banks used in phase
    A) and leave via a 256KB DMA per mt-pair (4KB contiguous rows); the
    final group uses quarter-evacs alternating V/S with a DMA each.
"""

import numpy as np

from concourse import bacc
import concourse.mybir as mybir
import concourse.tile as tile
from concourse.bass_utils import run_bass_kernel_spmd

# Problem shape (hardcoded per contract)
B, N, DIM, H, D = 2, 2048, 1024, 16, 64
NCORES = 8
HPC = H // 4            # 4 heads per core
CH = 512                # sequence chunk (matmul moving free dim)
NCH = N // CH           # 4
KO = DIM // 128         # 8 contraction tiles for the projections
P = 128

GS = 2.0 ** -6          # Gram scale: keeps prods/norm2 in fp16 range
GS_SQRT = 2.0 ** -3     # folded into w_out on the host
EPS2 = 1e-24 * GS       # rsqrt bias = (F.normalize eps)^2 * GS

N_WARM = 18             # PE warmup matmuls (512-free) during the DMA fill
N_FILL = 3              # PE filler matmuls covering the B-pipeline fill

F32 = mybir.dt.float32
F16 = mybir.dt.float16
MULT = mybir.AluOpType.mult
AFT = mybir.ActivationFunctionType


def _scalar_rsqrt(nc, out, in_, bias_val):
    """rinv = rsqrt(in + bias) on the scalar engine (one ACT pass).

    nc.scalar.activation() refuses AFT.Rsqrt wholesale; the HW table is
    fine for this kernel's accuracy budget (verified against the
    reference), so emit the InstActivation directly.
    """
    eng = nc.scalar
    bias_ap = nc.const_aps.scalar_like(bias_val, in_)
    ins = [
        eng.lower_ap(in_),
        eng.lower_ap(bias_ap),
        mybir.ImmediateValue(dtype=mybir.dt.float32, value=1.0),
        mybir.ImmediateValue(dtype=mybir.dt.float32, value=0.0),
    ]
    return eng.add_instruction(
        mybir.InstActivation(
            name=eng.bass.get_next_instruction_name(),
            func=AFT.Rsqrt,
            ins=ins,
            outs=[eng.lower_ap(out)],
        )
    )


def _build_bass():
    nc = bacc.Bacc("TRN2", target_bir_lowering=False, debug=False)

    # const for the rsqrt bias (mirrors Bass's const registration)
    _eps_t = nc.alloc_sbuf_tensor("const-float32-eps2", [128, 1], F32)
    nc.gpsimd.memset(_eps_t.ap(), EPS2)
    nc.const_aps.aps[(F32, EPS2)] = _eps_t.ap()
    # DMA-free warmup operand (the PE clock gate must open before the
    # first input DMA completes, which takes ~9us of ring bring-up)
    _warm_t = nc.alloc_sbuf_tensor("warmsrc", [128, 512], F16)
    nc.gpsimd.memset(_warm_t.ap(), 1.0)
    warmsrc = _warm_t.ap()

    # wq and x chunk 0 are packed into ONE tensor so the first q matmul
    # waits on a single DMA completion (each completion costs ~2.5us of
    # HBM round-trip latency on top of the stream time)
    wx0_d = nc.dram_tensor("wx0", [P, KO, 768], F16, kind="ExternalInput").ap()
    x_d = nc.dram_tensor("xt", [NCH - 1, P, KO, CH], F16, kind="ExternalInput").ap()
    wkv_d = nc.dram_tensor("wkv", [P, KO, 512], F16, kind="ExternalInput").ap()
    wo_d = nc.dram_tensor("wo", [P, 2, 1024], F16, kind="ExternalInput").ap()
    bo_d = nc.dram_tensor("bo", [P, 128], F16, kind="ExternalInput").ap()
    out_d = nc.dram_tensor("outT", [NCH, P, 8, CH], F16, kind="ExternalOutput").ap()

    with tile.TileContext(nc) as tc:
        with (
            tc.tile_pool(name="w", bufs=1) as wpool,
            tc.tile_pool(name="small", bufs=4) as smallpool,
            tc.tile_pool(name="stage", bufs=4) as stagepool,
            tc.tile_pool(name="ps", bufs=2, space="PSUM") as ps,
            tc.tile_pool(name="psg", bufs=3, space="PSUM") as psg,
        ):
            # ---- persistent tiles / input DMAs (one queue: all DMA
            # queues share a DRAM channel, so splitting queues only adds
            # ring bring-up latency) ----
            wx0 = wpool.tile([P, KO, 768], F16, name="wx0_sb")
            nc.sync.dma_start(wx0, wx0_d)
            x_all = wpool.tile([P, KO, NCH - 1, CH], F16, name="x_all")
            wkv = wpool.tile([P, KO, 512], F16, name="wkv_sb")
            nc.sync.dma_start(wkv, wkv_d)
            for cc in range(1, NCH):
                nc.sync.dma_start(x_all[:, :, cc - 1, :], x_d[cc - 1])
            wo = wpool.tile([P, 2, 1024], F16, name="wo_sb")
            nc.sync.dma_start(wo, wo_d)
            bo = wpool.tile([P, 128], F16, name="bo_sb")
            nc.sync.dma_start(bo, bo_d)

            qT = wpool.tile([P, 2, N], F16, name="qT_sb")       # [d-pair, n]
            kv = wpool.tile([P, 16, CH], F16, name="kv_sb")     # [n-tile, [k|v] per pair]
            outsc = wpool.tile([P, 2, N], F16, name="outsc_sb") # scaled out^T
            gwG_all = wpool.tile([P, 2, 128], F16, name="gwG_all")
            gwW_all = wpool.tile([P, 2, 128], F16, name="gwW_all")
            nc.gpsimd.memset(gwG_all, 0.0)
            nc.gpsimd.memset(gwW_all, 0.0)

            # PE warmup (no DMA dependency): opens the HAM clock gate
            # while the input rings bring up; results are discarded.
            warm_sb = smallpool.tile([P, 4], F32, name="warm_sb")
            ps_warm = psg.tile([P, 512], F32, tag="psg", name="ps_warm")
            for _ in range(N_WARM):
                nc.tensor.matmul(ps_warm, lhsT=warmsrc[:, 0:128], rhs=warmsrc,
                                 start=True, stop=True)
            nc.vector.tensor_copy(warm_sb, ps_warm[:, 0:4])

            # preload the scalar-engine Rsqrt table while the PE warms up
            rsq_warm = smallpool.tile([P, 1], F32, name="rsq_warm")
            _scalar_rsqrt(nc, rsq_warm, warm_sb[:, 0:1], EPS2)

            # Gram accumulators, each pinned in its own PSUM bank for all
            # of phase A (interleaved accumulation groups must not share
            # a bank: start=True clears the bank's has_written state).
            # Per pair: [:, 0:128] = k^T[k_e|k_o], [:, 128:256] =
            # k^T[v_e|v_o] (cross-head quadrants unused).
            gram0 = psg.tile([P, 512], F32, tag="psg", name="gram0")
            gram1 = psg.tile([P, 512], F32, tag="psg", name="gram1")
            gram = [gram0, gram1]

            # ---- phase A helpers ----
            def x_ap(c, ko):
                """x^T slice [P, CH] for chunk c, contraction tile ko."""
                if c == 0:
                    return wx0[:, ko, 256:768]
                return x_all[:, ko, c - 1, :]

            def q_one(c, mt, pool=None):
                ps_q = (pool or ps).tile(
                    [P, CH], F32,
                    tag="psg" if pool is not None else "ps", name="ps_q")
                for ko in range(KO):
                    nc.tensor.matmul(
                        ps_q,
                        lhsT=wx0[:, ko, mt * 128:(mt + 1) * 128],
                        rhs=x_ap(c, ko),
                        start=(ko == 0),
                        stop=(ko == KO - 1),
                    )
                return ps_q

            def q_evac(c, mt, ps_q):
                nc.vector.tensor_copy(qT[:, mt, c * CH:(c + 1) * CH], ps_q)

            def kv_group(c, nt):
                ps_kv = ps.tile([P, CH], F32, tag="ps", name="ps_kv")
                for ko in range(KO):
                    nc.tensor.matmul(
                        ps_kv,
                        lhsT=x_ap(c, ko)[:, nt * 128:(nt + 1) * 128],
                        rhs=wkv[:, ko, :],
                        start=(ko == 0),
                        stop=(ko == KO - 1),
                    )
                return ps_kv

            def kv_evac(c, nt, ps_kv, eng):
                dst = kv[:, c * 4 + nt, :]
                if eng == "v":
                    nc.vector.tensor_copy(dst, ps_kv)
                else:
                    nc.scalar.copy(dst, ps_kv)

            def gram_mms(c, nt):
                jt = c * 4 + nt
                for p in range(2):
                    blk = kv[:, jt, p * 256:(p + 1) * 256]
                    nc.tensor.matmul(
                        gram[p][:, 0:256], lhsT=blk[:, 0:128], rhs=blk,
                        start=(c == 0 and nt == 0),
                        stop=(c == 3 and nt == 3),
                    )

            # ---- phase A: chunks 0..1 (q first), chunks 2..3 (kv only;
            # their q groups are deferred past the Gram finalization so
            # real matmuls — not fillers — cover the B-pipeline fill) ----
            for c in range(2):
                ps_q0 = q_one(c, 0)
                ps_q1 = q_one(c, 1)
                if c > 0:
                    gram_mms(c - 1, 3)
                q_evac(c, 0, ps_q0)
                q_evac(c, 1, ps_q1)
                k = kv_group(c, 0)
                kv_evac(c, 0, k, "s")
                k = kv_group(c, 1)
                gram_mms(c, 0)
                kv_evac(c, 1, k, "v")
                k = kv_group(c, 2)
                gram_mms(c, 1)
                kv_evac(c, 2, k, "s")
                k = kv_group(c, 3)
                gram_mms(c, 2)
                kv_evac(c, 3, k, "v")

            for c in (2, 3):
                k = kv_group(c, 0)
                gram_mms(c - 1, 3)
                kv_evac(c, 0, k, "s")
                k = kv_group(c, 1)
                gram_mms(c, 0)
                kv_evac(c, 1, k, "v")
                k = kv_group(c, 2)
                gram_mms(c, 1)
                kv_evac(c, 2, k, "s")
                k = kv_group(c, 3)
                gram_mms(c, 2)
                kv_evac(c, 3, k, "v")

            ps_q0_ = q_one(2, 0)
            gram_mms(3, 3)  # final stop: Gram complete
            q_evac(2, 0, ps_q0_)
            # block-diagonal lhsT tiles (two heads stacked on K=128);
            # G gets the 2^-6 scale here (scalar ACT Copy with scale).
            for p in range(2):
                nc.scalar.mul(gwG_all[0:64, p, 0:64], gram[p][0:64, 0:64], GS)
                nc.scalar.mul(gwG_all[64:128, p, 64:128], gram[p][64:128, 64:128], GS)
                nc.vector.tensor_copy(gwW_all[0:64, p, 0:64], gram[p][0:64, 128:192])
                nc.vector.tensor_copy(gwW_all[64:128, p, 64:128], gram[p][64:128, 192:256])
            ps_q1_ = q_one(2, 1)
            q_evac(2, 1, ps_q1_)

            # ---- phase B/C, software-pipelined per chunk ----
            prods = {}
            lnr = {}
            stages = {}
            bts = {c: {} for c in range(NCH)}

            def t_mm(c):
                cs = slice(c * CH, (c + 1) * CH)
                res = []
                for p in range(2):
                    ps_t = ps.tile([P, CH], F32, tag="ps", name="ps_t")
                    nc.tensor.matmul(ps_t, lhsT=gwG_all[:, p, :],
                                     rhs=qT[:, p, cs], start=True, stop=True)
                    res.append(ps_t)
                bts[c]["t"] = res

            def prods_op(c):
                cs = slice(c * CH, (c + 1) * CH)
                for p in range(2):
                    prods[p, c] = stagepool.tile([P, CH], F16, name="prod",
                                                 tag="prod", bufs=4)
                    nc.vector.tensor_tensor(prods[p, c], bts[c]["t"][p],
                                            qT[:, p, cs], MULT)

            def rep_mm(c):
                res = []
                for p in range(2):
                    ps_r = ps.tile([P, CH], F32, tag="ps", name="ps_r")
                    nc.tensor.matmul(ps_r, lhsT=bo, rhs=prods[p, c],
                                     start=True, stop=True)
                    res.append(ps_r)
                bts[c]["rep"] = res

            def rsqrt_op(c):
                for p in range(2):
                    lnr[p, c] = stagepool.tile([P, CH], F16, name="lnr",
                                               tag="lnr", bufs=4)
                    _scalar_rsqrt(nc, lnr[p, c], bts[c]["rep"][p], EPS2)

            def o_mm(c):
                cs = slice(c * CH, (c + 1) * CH)
                res = []
                for p in range(2):
                    ps_o = ps.tile([P, CH], F32, tag="ps", name="ps_o")
                    nc.tensor.matmul(ps_o, lhsT=gwW_all[:, p, :],
                                     rhs=qT[:, p, cs], start=True, stop=True)
                    res.append(ps_o)
                bts[c]["o"] = res

            def mul_op(c):
                cs = slice(c * CH, (c + 1) * CH)
                for p in range(2):
                    nc.vector.tensor_tensor(outsc[:, p, cs], bts[c]["o"][p],
                                            lnr[p, c], MULT)

            def cg_group(c, g):
                """w_out partial for mt pair (2g, 2g+1) of chunk c."""
                cs = slice(c * CH, (c + 1) * CH)
                ps_cg = psg.tile([P, 1024], F32, tag="psg", name="ps_cg")
                for h in range(2):
                    mt = 2 * g + h
                    for kt in range(2):
                        nc.tensor.matmul(
                            ps_cg[:, h * 512:(h + 1) * 512],
                            lhsT=wo[:, kt, mt * 128:(mt + 1) * 128],
                            rhs=outsc[:, kt, cs],
                            start=(kt == 0),
                            stop=(kt == 1),
                        )
                return ps_cg

            def cg_evac(c, g, ps_cg, eng):
                if c not in stages:
                    stages[c] = stagepool.tile([P, 8, CH], F16, name="st",
                                               tag="st", bufs=3)
                dst = stages[c][:, 2 * g:2 * g + 2, :]
                if eng == "b":
                    # split across both engines (shortest latency; used
                    # on the last chunk to shrink the kernel tail)
                    nc.vector.tensor_copy(dst[:, 0, :], ps_cg[:, 0:512])
                    nc.scalar.copy(dst[:, 1, :], ps_cg[:, 512:1024])
                elif eng == "v":
                    nc.vector.tensor_copy(dst, ps_cg)
                else:
                    nc.scalar.copy(dst, ps_cg)

            def out_dma(c, g):
                nc.sync.dma_start(
                    out_d[c][:, 2 * g:2 * g + 2, :],
                    stages[c][:, 2 * g:2 * g + 2, :],
                )

            # B-pipeline fill for chunks 0 and 1.  PE fillers (on the
            # psg rotation, so they add no 'ps' slot pressure) are
            # distributed through the entry chain to absorb the V/S
            # latency the first chunks' matmuls would otherwise stall
            # on; they also keep the HAM gate open.
            def fill(n):
                for _ in range(n):
                    fw = psg.tile([P, 512], F32, tag="psg", name="fill")
                    nc.tensor.matmul(fw, lhsT=warmsrc[:, 0:128],
                                     rhs=warmsrc, start=True, stop=True)

            pq = q_one(3, 0)
            q_evac(3, 0, pq)
            t_mm(0)
            prods_op(0)
            pq = q_one(3, 1)
            q_evac(3, 1, pq)
            fill(1)
            rep_mm(0)
            rsqrt_op(0)
            o_mm(0)
            mul_op(0)
            fill(1)
            t_mm(1)
            fill(1)
            prods_op(1)
            fill(3)
            rep_mm(1)
            rsqrt_op(1)

            # steady state: cg(c) groups interleaved with B(c+2) stages.
            # V-queue order per iteration is mul(c+1), prods(c+2), then
            # evacs, so the next chunk's first cg group never waits.
            evac_eng = {0: "svss", 1: "svvs", 2: "svss", 3: "bbbb"}
            for c in range(NCH):
                pcg = cg_group(c, 0)
                cg_evac(c, 0, pcg, evac_eng[c][0])
                out_dma(c, 0)
                pcg = cg_group(c, 1)
                if c + 1 < NCH:
                    o_mm(c + 1)
                    mul_op(c + 1)
                if c + 2 < NCH:
                    t_mm(c + 2)
                    prods_op(c + 2)
                cg_evac(c, 1, pcg, evac_eng[c][1])
                out_dma(c, 1)
                pcg = cg_group(c, 2)
                cg_evac(c, 2, pcg, evac_eng[c][2])
                out_dma(c, 2)
                if c + 2 < NCH:
                    rep_mm(c + 2)
                    rsqrt_op(c + 2)
                pcg = cg_group(c, 3)
                if c == NCH - 1:
                    # final group: quarter evacs alternating V/S with a
                    # DMA per quarter — the shortest possible tail
                    st = stages[c]
                    for qq in range(4):
                        src = pcg[:, qq * 256:(qq + 1) * 256]
                        dstq = st[:, 6 + (qq // 2),
                                  (qq % 2) * 256:(qq % 2) * 256 + 256]
                        if qq % 2 == 0:
                            nc.vector.tensor_copy(dstq, src)
                        else:
                            nc.scalar.copy(dstq, src)
                        nc.sync.dma_start(
                            out_d[c][:, 6 + (qq // 2),
                                     (qq % 2) * 256:(qq % 2) * 256 + 256],
                            dstq)
                else:
                    cg_evac(c, 3, pcg, evac_eng[c][3])
                    out_dma(c, 3)

    nc.compile()
    return nc


_NC_CACHE = None


def _get_nc():
    global _NC_CACHE
    if _NC_CACHE is None:
        _NC_CACHE = _build_bass()
    return _NC_CACHE


def _build_in_maps(x, w_qkv, w_out_g):
    """Per-core device inputs (shared NEFF, different shards)."""
    bo = np.zeros((P, 128), np.float16)
    bo[0:64, 0:64] = 1.0
    bo[64:128, 64:128] = 1.0

    in_maps = []
    for core in range(NCORES):
        bi = core // 4
        hg = core % 4
        # x^T tiled chunk-major [nch, p, ko, ch]
        xt0 = x[bi].T.reshape(KO, P, N).transpose(1, 0, 2)
        xt = np.ascontiguousarray(
            np.stack([xt0[:, :, cc * CH:(cc + 1) * CH] for cc in range(NCH)]))
        # q rows of this head group, transposed -> [dim, 256] -> [p, ko, 256]
        rows_q = slice(hg * 256, hg * 256 + 256)
        wq = np.ascontiguousarray(
            w_qkv[rows_q].T.reshape(KO, P, 256).transpose(1, 0, 2))
        # per-head-pair [k_even | k_odd | v_even | v_odd] blocks
        blocks = []
        for pp in range(2):
            he = hg * HPC + 2 * pp
            ho = he + 1
            blocks.append(w_qkv[DIM + he * D: DIM + (he + 1) * D])
            blocks.append(w_qkv[DIM + ho * D: DIM + (ho + 1) * D])
            blocks.append(w_qkv[2 * DIM + he * D: 2 * DIM + (he + 1) * D])
            blocks.append(w_qkv[2 * DIM + ho * D: 2 * DIM + (ho + 1) * D])
        wkv_local = np.concatenate(blocks, axis=0)  # [512, dim]
        wkv = np.ascontiguousarray(
            wkv_local.T.reshape(KO, P, 512).transpose(1, 0, 2))
        # w_out columns for this head group (norm_g and the Gram-scale
        # compensation sqrt(GS)=0.125 folded), transposed
        wo_local = w_out_g[:, hg * 256:(hg + 1) * 256] * GS_SQRT  # [1024, 256]
        wo = np.ascontiguousarray(
            wo_local.T.reshape(2, P, 1024).transpose(1, 0, 2))
        xt16 = xt.astype(np.float16)
        # pack [wq | x chunk 0] per (p, ko) row -> one first DMA
        wx0 = np.ascontiguousarray(
            np.concatenate([wq.astype(np.float16), xt16[0]], axis=2))
        in_maps.append({
            "wx0": wx0, "xt": np.ascontiguousarray(xt16[1:]),
            "wkv": wkv.astype(np.float16), "wo": wo.astype(np.float16),
            "bo": bo,
        })
    return in_maps


def kernel(x, w_qkv, w_out, b_out, norm_g, norm_b):
    x = np.ascontiguousarray(np.asarray(x, dtype=np.float32))
    w_qkv = np.asarray(w_qkv, dtype=np.float32)
    w_out = np.asarray(w_out, dtype=np.float32)
    b_out = np.asarray(b_out, dtype=np.float32)
    g = np.asarray(norm_g, dtype=np.float32).reshape(H)
    bb = np.asarray(norm_b, dtype=np.float32).reshape(H)

    # Fold norm_g into w_out columns (attn scale per head passes through @v).
    w_out_g = w_out.copy()
    for h in range(H):
        w_out_g[:, h * D:(h + 1) * D] *= g[h]

    in_maps = _build_in_maps(x, w_qkv, w_out_g)

    nc = _get_nc()
    res = None
    last_exc = None
    for _attempt in range(3):
        try:
            res = run_bass_kernel_spmd(nc, in_maps, core_ids=list(range(NCORES)))
            break
        except Exception as e:  # transient NRT_EXEC_UNIT_UNRECOVERABLE etc.
            last_exc = e
            import time as _time
            _time.sleep(5)
    if res is None:
        raise last_exc

    out = np.zeros((B, N, DIM), np.float32)
    for core in range(NCORES):
        bi = core // 4
        r = res.results[core]["outT"].astype(np.float32)  # [NCH, P, 8, CH]
        partial = r.transpose(2, 1, 0, 3).reshape(DIM, N)
        out[bi] += partial.T
    out += b_out[None, None, :]

    # Exact rank-1 correction for norm_b (zero in practice).
    if np.any(bb != 0.0):
        for bi in range(B):
            corr = np.zeros(DIM, np.float64)
            for h in range(H):
                wv = w_qkv[2 * DIM + h * D: 2 * DIM + (h + 1) * D]  # [d, dim]
                vsum = (x[bi].astype(np.float64) @ wv.T.astype(np.float64)).sum(axis=0)
                # the +b term bypasses the g scale, so use the raw w_out
                corr += bb[h] * (w_out[:, h * D:(h + 1) * D].astype(np.float64) @ vsum)
            out[bi] += corr.astype(np.float32)[None, :]

    return out



# revision 8
# speedup vs baseline: 1.0085x; 1.0085x over previous
"""Trainium2 Bass kernel for the L2-normalized attention module.

Reference computation (per batch b):
    qkv = x @ w_qkv.T                        # [n, 3*dim]
    q,k,v per head h (d=64)                  # [n, d]
    dots = q @ k.T                           # [n, n]
    attn = dots / max(||dots_row||_2, eps) * g + b
    out_h = attn @ v                         # [n, d]
    final = concat_h(out_h) @ w_out.T + b_out

Key algebraic factorization: the score "nonlinearity" is only a per-row
scale r_i = 1/max(||dots_i||, eps), and ||dots_i||^2 = q_i^T (k^T k) q_i.
With W = k^T v, G = k^T k:
    outT[:, i] = r_i * (W^T q_i),   r_i = rsqrt(q_i^T G q_i)
so the n x n score matrix is never materialized.

Sharding: 8 cores = 2 batches x 4 head-groups (4 heads each).  Each core
computes its qkv slice, the factored attention for its 4 heads, and a
partial w_out projection; the host sums the 4 partials per batch (TP
reduction) and adds b_out.  norm_g (and a 2^-6 Gram scale compensation)
is folded into w_out on the host; norm_b (zero in practice) gets an
exact host-side rank-1 correction.

Schedule (single PE stream, authored for zero tensor-engine gaps so the
HAM clock gate never falls back to half rate):
  - DMA-free warmup matmuls (memset SBUF operand) open the PE clock
    gate during the ~9us before the first input DMA completes (template
    preamble + DMA fixed latency).
  - phase A per 512-seq chunk: q projection, then kv projection; the
    Gram [G|W] accumulates in per-pair pinned PSUM banks across the
    whole phase (no vector adds, no phase barrier; interleaved
    accumulation groups must NOT share a bank — start=True clears the
    bank's has_written state).  The last chunk computes kv first so the
    final Gram stop + block-diag copies overlap its q matmuls.
  - phase B/C is software-pipelined per chunk: the t/rep/o matmuls of
    chunk c+1 are emitted between the w_out matmul groups of chunk c,
    with the Vector/Scalar ops (prods, rsqrt, rinv-mul, PSUM
    evacuations) balanced to stay under the PE's per-chunk time.
  - rinv uses the scalar-engine Rsqrt table in ONE pass (instead of
    Ln+Exp): rinv = rsqrt(s*norm^2 + s*eps^2) with s=2^-6 keeping the
    fp16 intermediates in range; the 1/sqrt(s) factor is folded into
    w_out on the host.
  - w_out partials evacuate as [128,1024] two-bank PSUM reads (cg tiles
    rotate 3-deep through the same slots the Gram banks used in phase
    A) and leave via a 256KB DMA per mt-pair (4KB contiguous rows); the
    final group uses quarter-evacs alternating V/S with a DMA each.
"""

import numpy as np

from concourse import bacc
import concourse.mybir as mybir
import concourse.tile as tile
from concourse.bass_utils import run_bass_kernel_spmd

# Problem shape (hardcoded per contract)
B, N, DIM, H, D = 2, 2048, 1024, 16, 64
NCORES = 8
HPC = H // 4            # 4 heads per core
CH = 512                # sequence chunk (matmul moving free dim)
NCH = N // CH           # 4
KO = DIM // 128         # 8 contraction tiles for the projections
P = 128

GS = 2.0 ** -6          # Gram scale: keeps prods/norm2 in fp16 range
GS_SQRT = 2.0 ** -3     # folded into w_out on the host
EPS2 = 1e-24 * GS       # rsqrt bias = (F.normalize eps)^2 * GS

N_WARM = 7              # PE warmup matmuls (512-free) until the first wx0 slice lands
N_FILL = 3              # PE filler matmuls covering the B-pipeline fill

F32 = mybir.dt.float32
F16 = mybir.dt.float16
MULT = mybir.AluOpType.mult
AFT = mybir.ActivationFunctionType


def _scalar_rsqrt(nc, out, in_, bias_val):
    """rinv = rsqrt(in + bias) on the scalar engine (one ACT pass).

    nc.scalar.activation() refuses AFT.Rsqrt wholesale; the HW table is
    fine for this kernel's accuracy budget (verified against the
    reference), so emit the InstActivation directly.
    """
    eng = nc.scalar
    bias_ap = nc.const_aps.scalar_like(bias_val, in_)
    ins = [
        eng.lower_ap(in_),
        eng.lower_ap(bias_ap),
        mybir.ImmediateValue(dtype=mybir.dt.float32, value=1.0),
        mybir.ImmediateValue(dtype=mybir.dt.float32, value=0.0),
    ]
    return eng.add_instruction(
        mybir.InstActivation(
            name=eng.bass.get_next_instruction_name(),
            func=AFT.Rsqrt,
            ins=ins,
            outs=[eng.lower_ap(out)],
        )
    )


def _build_bass():
    nc = bacc.Bacc("TRN2", target_bir_lowering=False, debug=False)

    # DMA-free warmup operand (the PE clock gate must open before the
    # first input DMA completes; memset FIRST so the first warm matmul
    # issues as early as possible)
    _warm_t = nc.alloc_sbuf_tensor("warmsrc", [128, 512], F16)
    nc.gpsimd.memset(_warm_t.ap(), 1.0)
    warmsrc = _warm_t.ap()
    # const for the rsqrt bias (mirrors Bass's const registration)
    _eps_t = nc.alloc_sbuf_tensor("const-float32-eps2", [128, 1], F32)
    nc.gpsimd.memset(_eps_t.ap(), EPS2)
    nc.const_aps.aps[(F32, EPS2)] = _eps_t.ap()

    # wq and x chunk 0 are packed into ONE tensor so the first q matmul
    # waits on a single DMA completion (each completion costs ~2.5us of
    # HBM round-trip latency on top of the stream time)
    wx0_d = nc.dram_tensor("wx0", [P, KO, 768], F16, kind="ExternalInput").ap()
    x_d = nc.dram_tensor("xt", [NCH - 1, P, KO, CH], F16, kind="ExternalInput").ap()
    wkv_d = nc.dram_tensor("wkv", [P, KO, 512], F16, kind="ExternalInput").ap()
    wo_d = nc.dram_tensor("wo", [P, 2, 1024], F16, kind="ExternalInput").ap()
    bo_d = nc.dram_tensor("bo", [P, 128], F16, kind="ExternalInput").ap()
    out_d = nc.dram_tensor("outT", [NCH, P, 8, CH], F16, kind="ExternalOutput").ap()

    with tile.TileContext(nc) as tc:
        with (
            tc.tile_pool(name="w", bufs=1) as wpool,
            tc.tile_pool(name="small", bufs=4) as smallpool,
            tc.tile_pool(name="stage", bufs=4) as stagepool,
            tc.tile_pool(name="ps", bufs=2, space="PSUM") as ps,
            tc.tile_pool(name="psg", bufs=3, space="PSUM") as psg,
        ):
            # ---- persistent tiles / input DMAs (one queue: all DMA
            # queues share a DRAM channel, so splitting queues only adds
            # ring bring-up latency).  wx0 and wkv are split into 2-ko
            # slices so the first q matmuls can start on slice 0 (~1us
            # of stream) instead of waiting for the whole 1.5MiB wx0;
            # phase A then consumes at DMA pace with zero large stalls.
            wx0 = wpool.tile([P, KO, 768], F16, name="wx0_sb")
            for kp in range(4):
                nc.sync.dma_start(wx0[:, 2 * kp:2 * kp + 2, :],
                                  wx0_d[:, 2 * kp:2 * kp + 2, :])
            wkv = wpool.tile([P, KO, 512], F16, name="wkv_sb")
            for kp in range(4):
                nc.sync.dma_start(wkv[:, 2 * kp:2 * kp + 2, :],
                                  wkv_d[:, 2 * kp:2 * kp + 2, :])
            # chunk-major so each per-chunk DMA is contiguous per
            # partition (8KB rows -> one cheap descriptor, not 8)
            x_all = wpool.tile([P, NCH - 1, KO, CH], F16, name="x_all")
            for cc in range(1, NCH):
                nc.sync.dma_start(x_all[:, cc - 1, :, :], x_d[cc - 1])
            wo = wpool.tile([P, 2, 1024], F16, name="wo_sb")
            nc.sync.dma_start(wo, wo_d)
            bo = wpool.tile([P, 128], F16, name="bo_sb")
            nc.sync.dma_start(bo, bo_d)

            qT = wpool.tile([P, 2, N], F16, name="qT_sb")       # [d-pair, n]
            kv = wpool.tile([P, 16, CH], F16, name="kv_sb")     # [n-tile, [k|v] per pair]
            outsc = wpool.tile([P, 2, N], F16, name="outsc_sb") # scaled out^T
            gwG_all = wpool.tile([P, 2, 128], F16, name="gwG_all")
            gwW_all = wpool.tile([P, 2, 128], F16, name="gwW_all")
            nc.gpsimd.memset(gwG_all, 0.0)
            nc.gpsimd.memset(gwW_all, 0.0)

            # PE warmup (no DMA dependency): opens the HAM clock gate
            # while the input rings bring up; results are discarded.
            warm_sb = smallpool.tile([P, 4], F32, name="warm_sb")
            ps_warm = psg.tile([P, 512], F32, tag="psg", name="ps_warm")
            for _ in range(N_WARM):
                nc.tensor.matmul(ps_warm, lhsT=warmsrc[:, 0:128], rhs=warmsrc,
                                 start=True, stop=True)
            nc.vector.tensor_copy(warm_sb, ps_warm[:, 0:4])

            # preload the scalar-engine Rsqrt table while the PE warms up
            rsq_warm = smallpool.tile([P, 1], F32, name="rsq_warm")
            _scalar_rsqrt(nc, rsq_warm, warm_sb[:, 0:1], EPS2)

            # Gram accumulators, each pinned in its own PSUM bank for all
            # of phase A (interleaved accumulation groups must not share
            # a bank: start=True clears the bank's has_written state).
            # Per pair: [:, 0:128] = k^T[k_e|k_o], [:, 128:256] =
            # k^T[v_e|v_o] (cross-head quadrants unused).
            gram0 = psg.tile([P, 512], F32, tag="psg", name="gram0")
            gram1 = psg.tile([P, 512], F32, tag="psg", name="gram1")
            gram = [gram0, gram1]

            # ---- phase A helpers ----
            def x_ap(c, ko):
                """x^T slice [P, CH] for chunk c, contraction tile ko."""
                if c == 0:
                    return wx0[:, ko, 256:768]
                return x_all[:, c - 1, ko, :]

            def q_one(c, mt, pool=None):
                ps_q = (pool or ps).tile(
                    [P, CH], F32,
                    tag="psg" if pool is not None else "ps", name="ps_q")
                for ko in range(KO):
                    nc.tensor.matmul(
                        ps_q,
                        lhsT=wx0[:, ko, mt * 128:(mt + 1) * 128],
                        rhs=x_ap(c, ko),
                        start=(ko == 0),
                        stop=(ko == KO - 1),
                    )
                return ps_q

            def q_evac(c, mt, ps_q):
                nc.vector.tensor_copy(qT[:, mt, c * CH:(c + 1) * CH], ps_q)

            def kv_group(c, nt):
                ps_kv = ps.tile([P, CH], F32, tag="ps", name="ps_kv")
                for ko in range(KO):
                    nc.tensor.matmul(
                        ps_kv,
                        lhsT=x_ap(c, ko)[:, nt * 128:(nt + 1) * 128],
                        rhs=wkv[:, ko, :],
                        start=(ko == 0),
                        stop=(ko == KO - 1),
                    )
                return ps_kv

            def kv_evac(c, nt, ps_kv, eng):
                dst = kv[:, c * 4 + nt, :]
                if eng == "v":
                    nc.vector.tensor_copy(dst, ps_kv)
                else:
                    nc.scalar.copy(dst, ps_kv)

            def gram_mms(c, nt):
                jt = c * 4 + nt
                for p in range(2):
                    blk = kv[:, jt, p * 256:(p + 1) * 256]
                    nc.tensor.matmul(
                        gram[p][:, 0:256], lhsT=blk[:, 0:128], rhs=blk,
                        start=(c == 0 and nt == 0),
                        stop=(c == 3 and nt == 3),
                    )

            def q_first():
                """Chunk-0 q: both mt groups interleaved per 2-ko piece
                so the matmuls chase the 4 wx0 slice DMAs as they land
                (each piece = 4 matmuls ~ the stream time of the next
                piece)."""
                ps_q0 = ps.tile([P, CH], F32, tag="ps", name="ps_q")
                ps_q1 = ps.tile([P, CH], F32, tag="ps", name="ps_q")
                for kp in range(4):
                    for mt, psq in ((0, ps_q0), (1, ps_q1)):
                        for ko in (2 * kp, 2 * kp + 1):
                            nc.tensor.matmul(
                                psq,
                                lhsT=wx0[:, ko, mt * 128:(mt + 1) * 128],
                                rhs=x_ap(0, ko),
                                start=(ko == 0),
                                stop=(ko == KO - 1),
                            )
                return ps_q0, ps_q1

            # ---- phase A: chunks 0..1 (q first), chunks 2..3 (kv only;
            # their q groups are deferred past the Gram finalization so
            # real matmuls — not fillers — cover the B-pipeline fill) ----
            for c in range(2):
                if c == 0:
                    ps_q0, ps_q1 = q_first()
                else:
                    ps_q0 = q_one(c, 0)
                    ps_q1 = q_one(c, 1)
                if c > 0:
                    gram_mms(c - 1, 3)
                q_evac(c, 0, ps_q0)
                q_evac(c, 1, ps_q1)
                k = kv_group(c, 0)
                kv_evac(c, 0, k, "s")
                k = kv_group(c, 1)
                gram_mms(c, 0)
                kv_evac(c, 1, k, "v")
                k = kv_group(c, 2)
                gram_mms(c, 1)
                kv_evac(c, 2, k, "s")
                k = kv_group(c, 3)
                gram_mms(c, 2)
                kv_evac(c, 3, k, "v")

            for c in (2, 3):
                k = kv_group(c, 0)
                gram_mms(c - 1, 3)
                kv_evac(c, 0, k, "s")
                k = kv_group(c, 1)
                gram_mms(c, 0)
                kv_evac(c, 1, k, "v")
                k = kv_group(c, 2)
                gram_mms(c, 1)
                kv_evac(c, 2, k, "s")
                k = kv_group(c, 3)
                gram_mms(c, 2)
                kv_evac(c, 3, k, "v")

            ps_q0_ = q_one(2, 0)
            gram_mms(3, 3)  # final stop: Gram complete
            q_evac(2, 0, ps_q0_)
            # block-diagonal lhsT tiles (two heads stacked on K=128);
            # G gets the 2^-6 scale here (scalar ACT Copy with scale).
            for p in range(2):
                nc.scalar.mul(gwG_all[0:64, p, 0:64], gram[p][0:64, 0:64], GS)
                nc.scalar.mul(gwG_all[64:128, p, 64:128], gram[p][64:128, 64:128], GS)
                nc.vector.tensor_copy(gwW_all[0:64, p, 0:64], gram[p][0:64, 128:192])
                nc.vector.tensor_copy(gwW_all[64:128, p, 64:128], gram[p][64:128, 192:256])
            ps_q1_ = q_one(2, 1)
            q_evac(2, 1, ps_q1_)

            # ---- phase B/C, software-pipelined per chunk ----
            prods = {}
            lnr = {}
            stages = {}
            bts = {c: {} for c in range(NCH)}

            def t_mm(c):
                cs = slice(c * CH, (c + 1) * CH)
                res = []
                for p in range(2):
                    ps_t = ps.tile([P, CH], F32, tag="ps", name="ps_t")
                    nc.tensor.matmul(ps_t, lhsT=gwG_all[:, p, :],
                                     rhs=qT[:, p, cs], start=True, stop=True)
                    res.append(ps_t)
                bts[c]["t"] = res

            def prods_op(c):
                cs = slice(c * CH, (c + 1) * CH)
                for p in range(2):
                    prods[p, c] = stagepool.tile([P, CH], F16, name="prod",
                                                 tag="prod", bufs=4)
                    nc.vector.tensor_tensor(prods[p, c], bts[c]["t"][p],
                                            qT[:, p, cs], MULT)

            def rep_mm(c):
                res = []
                for p in range(2):
                    ps_r = ps.tile([P, CH], F32, tag="ps", name="ps_r")
                    nc.tensor.matmul(ps_r, lhsT=bo, rhs=prods[p, c],
                                     start=True, stop=True)
                    res.append(ps_r)
                bts[c]["rep"] = res

            def rsqrt_op(c):
                for p in range(2):
                    lnr[p, c] = stagepool.tile([P, CH], F16, name="lnr",
                                               tag="lnr", bufs=4)
                    _scalar_rsqrt(nc, lnr[p, c], bts[c]["rep"][p], EPS2)

            def o_mm(c):
                cs = slice(c * CH, (c + 1) * CH)
                res = []
                for p in range(2):
                    ps_o = ps.tile([P, CH], F32, tag="ps", name="ps_o")
                    nc.tensor.matmul(ps_o, lhsT=gwW_all[:, p, :],
                                     rhs=qT[:, p, cs], start=True, stop=True)
                    res.append(ps_o)
                bts[c]["o"] = res

            def mul_op(c):
                cs = slice(c * CH, (c + 1) * CH)
                for p in range(2):
                    nc.vector.tensor_tensor(outsc[:, p, cs], bts[c]["o"][p],
                                            lnr[p, c], MULT)

            def cg_group(c, g):
                """w_out partial for mt pair (2g, 2g+1) of chunk c."""
                cs = slice(c * CH, (c + 1) * CH)
                ps_cg = psg.tile([P, 1024], F32, tag="psg", name="ps_cg")
                for h in range(2):
                    mt = 2 * g + h
                    for kt in range(2):
                        nc.tensor.matmul(
                            ps_cg[:, h * 512:(h + 1) * 512],
                            lhsT=wo[:, kt, mt * 128:(mt + 1) * 128],
                            rhs=outsc[:, kt, cs],
                            start=(kt == 0),
                            stop=(kt == 1),
                        )
                return ps_cg

            def cg_evac(c, g, ps_cg, eng):
                if c not in stages:
                    stages[c] = stagepool.tile([P, 8, CH], F16, name="st",
                                               tag="st", bufs=3)
                dst = stages[c][:, 2 * g:2 * g + 2, :]
                if eng == "b":
                    # split across both engines (shortest latency; used
                    # on the last chunk to shrink the kernel tail)
                    nc.vector.tensor_copy(dst[:, 0, :], ps_cg[:, 0:512])
                    nc.scalar.copy(dst[:, 1, :], ps_cg[:, 512:1024])
                elif eng == "v":
                    nc.vector.tensor_copy(dst, ps_cg)
                else:
                    nc.scalar.copy(dst, ps_cg)

            # B-pipeline fill for chunks 0 and 1.  PE fillers (on the
            # psg rotation, so they add no 'ps' slot pressure) are
            # distributed through the entry chain to absorb the V/S
            # latency the first chunks' matmuls would otherwise stall
            # on; they also keep the HAM gate open.
            def fill(n):
                for _ in range(n):
                    fw = psg.tile([P, 512], F32, tag="psg", name="fill")
                    nc.tensor.matmul(fw, lhsT=warmsrc[:, 0:128],
                                     rhs=warmsrc, start=True, stop=True)

            pq = q_one(3, 0)
            q_evac(3, 0, pq)
            t_mm(0)
            prods_op(0)
            pq = q_one(3, 1)
            q_evac(3, 1, pq)
            fill(1)
            rep_mm(0)
            rsqrt_op(0)
            o_mm(0)
            mul_op(0)
            fill(1)
            t_mm(1)
            fill(1)
            prods_op(1)
            fill(3)
            rep_mm(1)
            rsqrt_op(1)

            # steady state: cg(c) groups interleaved with B(c+2) stages.
            # V-queue order per iteration is mul(c+1), prods(c+2), then
            # evacs, so the next chunk's first cg group never waits.
            # Chunks 0..2 ship via ONE 1MiB DMA each (each DMA_DIRECT2D
            # dispatch costs ~600ns of serialized Sync time; per-group
            # DMAs back the Sync queue up into the kernel tail).  Chunk
            # 3 splits: groups 0-1 and 2 go out as soon as evac'd, the
            # final group as two half evacs on V and S in parallel, each
            # with its own small DMA — the shortest possible tail.
            evac_eng = {0: "svvs", 1: "svvs", 2: "svvs", 3: "bbbb"}
            for c in range(NCH):
                pcg = cg_group(c, 0)
                cg_evac(c, 0, pcg, evac_eng[c][0])
                pcg = cg_group(c, 1)
                if c + 1 < NCH:
                    o_mm(c + 1)
                    mul_op(c + 1)
                if c + 2 < NCH:
                    t_mm(c + 2)
                    prods_op(c + 2)
                cg_evac(c, 1, pcg, evac_eng[c][1])
                if c == NCH - 1:
                    nc.sync.dma_start(out_d[c][:, 0:4, :],
                                      stages[c][:, 0:4, :])
                pcg = cg_group(c, 2)
                cg_evac(c, 2, pcg, evac_eng[c][2])
                if c == NCH - 1:
                    nc.sync.dma_start(out_d[c][:, 4:6, :],
                                      stages[c][:, 4:6, :])
                if c + 2 < NCH:
                    rep_mm(c + 2)
                    rsqrt_op(c + 2)
                pcg = cg_group(c, 3)
                if c == NCH - 1:
                    # final group: two half evacs in parallel (V and S
                    # are both free here), a DMA right behind each
                    st = stages[c]
                    nc.vector.tensor_copy(st[:, 6, :], pcg[:, 0:512])
                    nc.sync.dma_start(out_d[c][:, 6, :], st[:, 6, :])
                    nc.scalar.copy(st[:, 7, :], pcg[:, 512:1024])
                    nc.sync.dma_start(out_d[c][:, 7, :], st[:, 7, :])
                else:
                    cg_evac(c, 3, pcg, evac_eng[c][3])
                    nc.sync.dma_start(out_d[c], stages[c])

    nc.compile()
    return nc


_NC_CACHE = None


def _get_nc():
    global _NC_CACHE
    if _NC_CACHE is None:
        _NC_CACHE = _build_bass()
    return _NC_CACHE


def _build_in_maps(x, w_qkv, w_out_g):
    """Per-core device inputs (shared NEFF, different shards)."""
    bo = np.zeros((P, 128), np.float16)
    bo[0:64, 0:64] = 1.0
    bo[64:128, 64:128] = 1.0

    in_maps = []
    for core in range(NCORES):
        bi = core // 4
        hg = core % 4
        # x^T tiled chunk-major [nch, p, ko, ch]
        xt0 = x[bi].T.reshape(KO, P, N).transpose(1, 0, 2)
        xt = np.ascontiguousarray(
            np.stack([xt0[:, :, cc * CH:(cc + 1) * CH] for cc in range(NCH)]))
        # q rows of this head group, transposed -> [dim, 256] -> [p, ko, 256]
        rows_q = slice(hg * 256, hg * 256 + 256)
        wq = np.ascontiguousarray(
            w_qkv[rows_q].T.reshape(KO, P, 256).transpose(1, 0, 2))
        # per-head-pair [k_even | k_odd | v_even | v_odd] blocks
        blocks = []
        for pp in range(2):
            he = hg * HPC + 2 * pp
            ho = he + 1
            blocks.append(w_qkv[DIM + he * D: DIM + (he + 1) * D])
            blocks.append(w_qkv[DIM + ho * D: DIM + (ho + 1) * D])
            blocks.append(w_qkv[2 * DIM + he * D: 2 * DIM + (he + 1) * D])
            blocks.append(w_qkv[2 * DIM + ho * D: 2 * DIM + (ho + 1) * D])
        wkv_local = np.concatenate(blocks, axis=0)  # [512, dim]
        wkv = np.ascontiguousarray(
            wkv_local.T.reshape(KO, P, 512).transpose(1, 0, 2))
        # w_out columns for this head group (norm_g and the Gram-scale
        # compensation sqrt(GS)=0.125 folded), transposed
        wo_local = w_out_g[:, hg * 256:(hg + 1) * 256] * GS_SQRT  # [1024, 256]
        wo = np.ascontiguousarray(
            wo_local.T.reshape(2, P, 1024).transpose(1, 0, 2))
        xt16 = xt.astype(np.float16)
        # pack [wq | x chunk 0] per (p, ko) row -> one first DMA
        wx0 = np.ascontiguousarray(
            np.concatenate([wq.astype(np.float16), xt16[0]], axis=2))
        in_maps.append({
            "wx0": wx0, "xt": np.ascontiguousarray(xt16[1:]),
            "wkv": wkv.astype(np.float16), "wo": wo.astype(np.float16),
            "bo": bo,
        })
    return in_maps


def kernel(x, w_qkv, w_out, b_out, norm_g, norm_b):
    x = np.ascontiguousarray(np.asarray(x, dtype=np.float32))
    w_qkv = np.asarray(w_qkv, dtype=np.float32)
    w_out = np.asarray(w_out, dtype=np.float32)
    b_out = np.asarray(b_out, dtype=np.float32)
    g = np.asarray(norm_g, dtype=np.float32).reshape(H)
    bb = np.asarray(norm_b, dtype=np.float32).reshape(H)

    # Fold norm_g into w_out columns (attn scale per head passes through @v).
    w_out_g = w_out.copy()
    for h in range(H):
        w_out_g[:, h * D:(h + 1) * D] *= g[h]

    in_maps = _build_in_maps(x, w_qkv, w_out_g)

    nc = _get_nc()
    res = None
    last_exc = None
    for _attempt in range(3):
        try:
            res = run_bass_kernel_spmd(nc, in_maps, core_ids=list(range(NCORES)))
            break
        except Exception as e:  # transient NRT_EXEC_UNIT_UNRECOVERABLE etc.
            last_exc = e
            import time as _time
            _time.sleep(5)
    if res is None:
        raise last_exc

    out = np.zeros((B, N, DIM), np.float32)
    for core in range(NCORES):
        bi = core // 4
        r = res.results[core]["outT"].astype(np.float32)  # [NCH, P, 8, CH]
        partial = r.transpose(2, 1, 0, 3).reshape(DIM, N)
        out[bi] += partial.T
    out += b_out[None, None, :]

    # Exact rank-1 correction for norm_b (zero in practice).
    if np.any(bb != 0.0):
        for bi in range(B):
            corr = np.zeros(DIM, np.float64)
            for h in range(H):
                wv = w_qkv[2 * DIM + h * D: 2 * DIM + (h + 1) * D]  # [d, dim]
                vsum = (x[bi].astype(np.float64) @ wv.T.astype(np.float64)).sum(axis=0)
                # the +b term bypasses the g scale, so use the raw w_out
                corr += bb[h] * (w_out[:, h * D:(h + 1) * D].astype(np.float64) @ vsum)
            out[bi] += corr.astype(np.float32)[None, :]

    return out

